# revision 18
# baseline (speedup 1.0000x reference)
"""Causal self-attention (B=4,T=2048,C=1024) on 8 TRN2 NeuronCores.

Sharding: core c = 2*b + h handles batch b and global q-blocks g = 2k+h
(k=0..7, 128 rows each). K/V projection work is split across the pair:
core h projects kv only for s-block pairs (4r+2h, 4r+2h+1), r=0..3.
Pairs are exchanged with 3 pairwise AllGather rounds (DRAM bounce)
covering s-blocks 0..7 / 8..11 / 12..15; each round has ~15us fixed
cost on the CC core, so the first round is double-size.  The rounds
ride under the staging + q-projection compute and land (with
need-ordered readbacks) just before attention consumes them.

Engine roles are kept disjoint so the Tile scheduler cannot interleave
a CC-completion wait in front of a later CC trigger:
  gpsimd: wk/xT per-cb slab loads (2KB lines), snd DMAs, CC triggers.
  sync:   wv/wq/xq/mask loads, then per-round readbacks of rcv.
  scalar: PSUM drains, exp, y DMA posts.
  vector: attention probsT copies / reductions.
Attention runs q-blocks ascending with k=0 moved last (small final
tail); the only work depending on the last exchange round is the tail
of k=6/k=7, ~40us after the round completes.  Softmax skips
max-subtraction (scores are bounded ~2.5) and blocks are software-
pipelined (scores of block i+1 run between scores and AV of block i).
"""

import math
import sys

for p in ("/opt/trn_rl_repo",):
    if p not in sys.path:
        sys.path.insert(0, p)

import numpy as np
import ml_dtypes

import concourse.bass as bass
import concourse.tile as tile
from concourse import mybir
from concourse.masks import make_identity
from concourse.bass_utils import run_bass_kernel_spmd

B, T, C = 4, 2048, 1024
P = 128
NQB = 8            # q-blocks per core
NCB = C // P       # 8 c-chunks (contraction for projections)
NDB = C // P       # 8 d-chunks (contraction for QK)
NSB = T // P       # 16 s-blocks
F32 = mybir.dt.float32
BF16 = mybir.dt.bfloat16
SCALE = 1.0 / math.sqrt(C)
NEG = -1e30
GROUPS = [[0, 1], [2, 3], [4, 5], [6, 7]]
ROUNDS = [(0,), (1,), (2,), (3,)]   # local-pair grouping per exchange round


def build_nc(jitter=0):
    nc = bass.Bass(num_devices=8)
    xT = nc.declare_dram_parameter("xT", [C, T // 2], BF16, isOutput=False)
    xq = nc.declare_dram_parameter("xq", [C, NQB * P], BF16, isOutput=False)
    w = nc.declare_dram_parameter("w", [C, 3 * C], BF16, isOutput=False)
    mask = nc.declare_dram_parameter("mask", [P, 2 * P], BF16, isOutput=False)
    out = nc.declare_dram_parameter("out", [NQB * P, C], BF16, isOutput=True)

    audit = {"cc": [], "rb": [], "load": [], "yout": []}

    from contextlib import ExitStack
    with tile.TileContext(nc) as tc, ExitStack() as ctx:
        singles = ctx.enter_context(tc.tile_pool(name="singles", bufs=1))
        xqpool = ctx.enter_context(tc.tile_pool(name="xqpool", bufs=1))
        xtpool = ctx.enter_context(tc.tile_pool(name="xtpool", bufs=1))
        wbuf = ctx.enter_context(tc.tile_pool(name="wbuf", bufs=1))
        qkv = ctx.enter_context(tc.tile_pool(name="qkv", bufs=1))
        att = ctx.enter_context(tc.tile_pool(name="att", bufs=4))
        attT = ctx.enter_context(tc.tile_pool(name="attT", bufs=4))
        ybuf = ctx.enter_context(tc.tile_pool(name="ybuf", bufs=2))
        stat = ctx.enter_context(tc.tile_pool(name="stat", bufs=4))
        psA = ctx.enter_context(tc.tile_pool(name="psA", bufs=4, space="PSUM"))
        psT = ctx.enter_context(tc.tile_pool(name="psT", bufs=2, space="PSUM"))
        psY = ctx.enter_context(tc.tile_pool(name="psY", bufs=2, space="PSUM"))
        dram = ctx.enter_context(tc.tile_pool(name="dram", bufs=10,
                                              space="DRAM"))

        ident = singles.tile([P, P], BF16)
        make_identity(nc, ident)

        touch_scr = stat.tile([P, 2], F32, tag="touch")
        for _ in range(jitter):  # schedule perturbation for wait-audit retries
            nc.vector.tensor_copy(out=touch_scr, in_=touch_scr)

        # resident weights / activations; DMA issue order = need order.
        wk_all = wbuf.tile([P, NCB, C], BF16, tag="wk_all")
        wv_all = wbuf.tile([P, NCB, C], BF16, tag="wv_all")
        wq_all = wbuf.tile([P, NCB, C], BF16, tag="wq_all")
        xT_sb = xtpool.tile([P, NCB, T // 2], BF16, tag="xT")
        xq_sb = xqpool.tile([P, NCB, NQB * P], BF16, tag="xq")
        mask_sb = singles.tile([P, 2 * P], BF16)

        # Split the load bandwidth across the two DMA queues in first-need
        # order: gpsimd carries wk slabs then wv (K then V projection
        # inputs), sync carries xT slabs then wq/xq.  All 2KB lines.
        for cb in range(NCB):
            audit["load"].append(nc.gpsimd.dma_start(
                out=wk_all[:, cb, :], in_=w[cb * P:(cb + 1) * P, C:2 * C]))
            audit["load"].append(nc.sync.dma_start(
                out=xT_sb[:, cb, :], in_=xT[cb * P:(cb + 1) * P, :]))
        audit["load"].append(nc.gpsimd.dma_start(
            out=wv_all,
            in_=w[:, 2 * C:3 * C].rearrange("(cb p) d -> p cb d", p=P)))
        audit["load"].append(nc.sync.dma_start(
            out=wq_all, in_=w[:, 0:C].rearrange("(cb p) d -> p cb d", p=P)))
        audit["load"].append(nc.sync.dma_start(
            out=xq_sb, in_=xq[:, :].rearrange("(cb p) t -> p cb t", p=P)))
        audit["load"].append(nc.sync.dma_start(out=mask_sb, in_=mask[:, :]))

        # persistent SBUF tensors
        qT_sb = qkv.tile([P, NDB, NQB * P], BF16)   # [d%128, d//128, t]  2MB
        kT_sb = qkv.tile([P, NDB, T], BF16)         # [d%128, d//128, s]  4MB
        v_sb = qkv.tile([P, NSB, C], BF16)          # [s%128, s//128, d]  4MB

        # ---------------- Phase KV + exchange -----------------------------
        # Local pair r (xT_sb cols 256r..256r+256) is staged at the slot-0
        # position of its round: kT_sb cols 512r..512r+256 / v_sb blocks
        # (4r, 4r+1).  The round's AllGather then lands slot h -> cols
        # 512r+256h..512r+256(h+1) / v blocks (4r+2h, 4r+2h+1), making
        # every kT/v region rb-written and final in global s order.
        deferred_rb = []
        for prs in ROUNDS:
            npair = len(prs)
            for db in range(NDB):
                ps = psA.tile([P, 512], F32, tag="ps")
                wd = npair * 2 * P
                xo = prs[0] * 2 * P
                for cb in range(NCB):
                    nc.tensor.matmul(
                        ps[:, 0:wd], wk_all[:, cb, db * P:(db + 1) * P],
                        xT_sb[:, cb, xo:xo + wd],
                        start=(cb == 0), stop=(cb == NCB - 1))
                for i, pr in enumerate(prs):
                    nc.scalar.copy(
                        out=kT_sb[:, db, 4 * pr * P:(4 * pr + 2) * P],
                        in_=ps[:, 2 * i * P:2 * (i + 1) * P])
            for pr in prs:
                for i, sblk in enumerate((4 * pr, 4 * pr + 1)):
                    ps0 = psA.tile([P, 512], F32, tag="ps")
                    ps1 = psA.tile([P, 512], F32, tag="ps")
                    xc = pr * 2 * P + i * P
                    for cb in range(NCB):
                        nc.tensor.matmul(
                            ps0, xT_sb[:, cb, xc:xc + P],
                            wv_all[:, cb, 0:512],
                            start=(cb == 0), stop=(cb == NCB - 1))
                        nc.tensor.matmul(
                            ps1, xT_sb[:, cb, xc:xc + P],
                            wv_all[:, cb, 512:1024],
                            start=(cb == 0), stop=(cb == NCB - 1))
                    nc.scalar.copy(out=v_sb[:, sblk, 0:512], in_=ps0)
                    nc.scalar.copy(out=v_sb[:, sblk, 512:1024], in_=ps1)

            # exchange round: snd = [kT(pair) for pair] + [v(pair) for pair]
            slot = npair * 4096          # bf16 elements per partition
            snd = dram.tile([P, slot], BF16, tag=f"snd{prs[0]}")
            rcv = dram.tile([2 * P * slot], BF16, tag=f"rcv{prs[0]}")
            for i, pr in enumerate(prs):
                nc.gpsimd.dma_start(
                    out=snd[:, i * 2048:(i + 1) * 2048],
                    in_=kT_sb[:, :, 4 * pr * P:(4 * pr + 2) * P])
            for i, pr in enumerate(prs):
                nc.gpsimd.dma_start(
                    out=snd[:, (npair + i) * 2048:(npair + i + 1) * 2048],
                    in_=v_sb[:, 4 * pr:4 * pr + 2, :])
            cc = nc.gpsimd.collective_compute(
                "AllGather", mybir.AluOpType.bypass, replica_groups=GROUPS,
                ins=[snd[:, :].rearrange("p n -> (p n)")],
                outs=[rcv[:]])
            audit["cc"].append(cc)

            def rb(prs=prs, npair=npair, slot=slot, rcv=rcv):
                # need-ordered: kT of both slots first, then v of both
                # slots, earliest s-blocks first.
                saps = [rcv[s * P * slot:(s + 1) * P * slot].rearrange(
                    "(p c h) -> p c h", p=P, c=2 * npair) for s in range(2)]
                for i, pr in enumerate(prs):
                    for s in range(2):
                        co = (4 * pr + 2 * s) * P
                        audit["rb"].append(nc.sync.dma_start(
                            out=kT_sb[:, :, co:co + 2 * P],
                            in_=saps[s][:, i, :]))
                for i, pr in enumerate(prs):
                    for s in range(2):
                        audit["rb"].append(nc.sync.dma_start(
                            out=v_sb[:, 4 * pr + 2 * s:4 * pr + 2 * s + 2,
                                     :],
                            in_=saps[s][:, npair + i, :]))
            deferred_rb.append(rb)

        # all CC triggers posted; now post the readbacks (sync engine, so a
        # readback waiting on a CC-completion semaphore can never delay a
        # later trigger on the gpsimd stream)
        for rb_fn in deferred_rb:
            rb_fn()

        # ---------------- Phase Q: qT = (W_q^T @ xq) * scale --------------
        for th in range(2):
            for db in range(NDB):
                ps = psA.tile([P, 512], F32, tag="ps")
                for cb in range(NCB):
                    nc.tensor.matmul(
                        ps, wq_all[:, cb, db * P:(db + 1) * P],
                        xq_sb[:, cb, th * 512:(th + 1) * 512],
                        start=(cb == 0), stop=(cb == NCB - 1))
                nc.scalar.mul(
                    out=qT_sb[:, db, th * 512:(th + 1) * 512], in_=ps,
                    mul=SCALE)

        # ---------------- Phase ATT (software pipelined) ------------------
        # Unit schedule (S_k = scores of q-block k, split into 512-col
        # chunks where useful; T_k = transpose+AV tail; X_k = early
        # transposes):
        #   S1 | S0 T1 | S2 T0 | S3 T2 | S7a T3 | S4 X7 | S5 T4
        #   | S6a T5 | X6 S7b S6b | T7 | T6
        # S0/S7a need only s-blocks 0..7 and S6a only 0..11, so the only
        # work gated on the last exchange rounds sits at the very end.
        state = {}

        def emit_scores(k, chlo, chhi):
            L = 2 * k + 2
            cols = L * P
            nch = (cols + 511) // 512
            if k not in state:
                state[k] = {
                    "probs": att.tile([P, NQB * 2 * P], BF16, tag="probs",
                                      name=f"probs{k}"),
                    "sums": stat.tile([P, 8], F32, tag="sums",
                                      name=f"sums{k}"),
                    "probsT": None, "nch": nch,
                }
            st = state[k]
            lo = cols - 256
            ch0, off = divmod(lo, 512)
            for ch in range(chlo, min(chhi, nch)):
                wd = min(512, cols - ch * 512)
                ps = psA.tile([P, 512], F32, tag="ps")
                has_mask = ch == ch0
                for db in range(NDB):
                    nc.tensor.matmul(
                        ps[:, 0:wd], qT_sb[:, db, k * P:(k + 1) * P],
                        kT_sb[:, db, ch * 512:ch * 512 + wd],
                        start=(db == 0),
                        stop=(not has_mask and db == NDB - 1))
                if has_mask:
                    # mask folded into the accumulation group
                    nc.tensor.matmul(
                        ps[:, off:off + 256], ident, mask_sb,
                        start=False, stop=True)
                # no max-subtraction: |score| <= ~2.5, exp is safe
                nc.scalar.activation(
                    out=st["probs"][:, ch * 512:ch * 512 + wd],
                    in_=ps[:, 0:wd],
                    func=mybir.ActivationFunctionType.Exp,
                    bias=0.0, scale=1.0,
                    accum_out=st["sums"][:, ch:ch + 1])

        def emit_transp(k, jlo, jhi):
            st = state[k]
            if st["probsT"] is None:
                st["probsT"] = attT.tile([P, NQB * 2, P], BF16, tag="probsT",
                                         name=f"probsT{k}")
            probs, probsT = st["probs"], st["probsT"]
            for j in range(jlo, jhi, 4):
                g = min(4, jhi - j)
                pt = psT.tile([P, 4 * P], BF16)
                for jj in range(g):
                    nc.tensor.transpose(
                        pt[:, jj * P:(jj + 1) * P],
                        probs[:, (j + jj) * P:(j + jj + 1) * P], ident)
                nc.vector.tensor_copy(out=probsT[:, j:j + g, :],
                                      in_=pt[:, 0:g * P])

        def emit_av(k):
            L = 2 * k + 2
            st = state[k]
            probsT = st["probsT"]
            rsum = stat.tile([P, 1], F32, tag="rsum")
            nc.vector.reduce_sum(
                out=rsum, in_=st["sums"][:, 0:st["nch"]],
                axis=mybir.AxisListType.X)
            recip = stat.tile([P, 1], F32, tag="recip")
            nc.vector.reciprocal(out=recip, in_=rsum)
            y_sb = ybuf.tile([P, C], BF16, tag="y")
            for dh in range(2):
                py = psY.tile([P, 512], F32, tag="py")
                for j in range(L):
                    nc.tensor.matmul(
                        py, probsT[:, j, :],
                        v_sb[:, j, dh * 512:(dh + 1) * 512],
                        start=(j == 0), stop=(j == L - 1))
                nc.scalar.activation(
                    out=y_sb[:, dh * 512:(dh + 1) * 512], in_=py,
                    func=mybir.ActivationFunctionType.Copy, bias=0.0,
                    scale=recip)
            audit["yout"].append(nc.scalar.dma_start(
                out=out[k * P:(k + 1) * P, :], in_=y_sb))

        def tail(k):
            emit_transp(k, 0, 2 * k + 2)
            emit_av(k)

        emit_scores(1, 0, 4)
        emit_scores(0, 0, 4); tail(1)
        emit_scores(2, 0, 4); tail(0)
        emit_scores(3, 0, 4); tail(2)
        emit_scores(7, 0, 2); tail(3)           # k7 cols 0:1024 (s0..7)
        emit_scores(4, 0, 4); emit_transp(7, 0, 8)
        emit_scores(5, 0, 4); tail(4)
        emit_scores(6, 0, 3); tail(5)           # k6 cols 0:1536 (s0..11)
        emit_transp(6, 0, 8)
        emit_scores(7, 2, 4)                    # k7 cols 1024:2048
        emit_scores(6, 3, 4)                    # k6 cols 1536:1792
        emit_transp(7, 8, 16); emit_av(7)
        emit_transp(6, 8, 14); emit_av(6)

    return nc, audit


def _host_inputs(x, W):
    """Build per-core input maps."""
    tril = np.where(
        np.arange(P)[None, :] <= np.arange(P)[:, None], 0.0, NEG
    ).astype(np.float32)
    mask_even = np.concatenate([tril, np.full((P, P), NEG, np.float32)], 1)
    mask_odd = np.concatenate([np.zeros((P, P), np.float32), tril], 1)
    in_maps = []
    for c in range(8):
        b, h = divmod(c, 2)
        xb = x[b].astype(ml_dtypes.bfloat16)        # [T, C]
        # local kv token columns: pairs (4r+2h, 4r+2h+1), r=0..3
        kvrows = np.concatenate(
            [np.arange((4 * r + 2 * h) * P, (4 * r + 2 * h + 2) * P)
             for r in range(4)])
        xTh = np.ascontiguousarray(xb[kvrows].T)    # [C, 1024]
        qrows = np.concatenate(
            [np.arange((2 * k + h) * P, (2 * k + h + 1) * P)
             for k in range(NQB)])
        xq = np.ascontiguousarray(xb[qrows].T)      # [C, 1024]
        in_maps.append({
            "xT": xTh, "xq": xq, "w": W.astype(ml_dtypes.bfloat16),
            "mask": (mask_even if h == 0 else mask_odd).astype(
                ml_dtypes.bfloat16),
        })
    return in_maps


def _gather(results):
    y = np.zeros((B, T, C), np.float32)
    for c in range(8):
        b, h = divmod(c, 2)
        yc = results[c]["out"]
        for k in range(NQB):
            g = 2 * k + h
            y[b, g * P:(g + 1) * P, :] = yc[k * P:(k + 1) * P, :]
    return y


_SKIP_TYPES = ("InstCall", "InstUnconditionalBranch")


def _wait_limit(inst):
    t = type(inst).__name__
    if t in _SKIP_TYPES:
        return None
    return 1


def _split_excess_waits(nc):
    """HW instruction structs carry few sync-wait slots (1 for compute,
    2 for pseudo-DMA). Move excess waits onto same-engine EventSemaphore
    instructions inserted just before the offender (engines execute their
    stream in order, so this preserves semantics)."""
    fix = 0
    for blk in nc.m.functions[0].blocks:
        out = []
        for inst in blk.instructions:
            lim = _wait_limit(inst)
            si = inst.sync_info
            waits = list(si.on_wait) if si and si.on_wait else []
            if lim is not None and len(waits) > lim:
                for w in waits[:-lim]:
                    fix += 1
                    e = mybir.InstEventSemaphore(
                        name=f"I-waitfix-{fix}", ins=[], outs=[],
                        sync_info=mybir.SyncInfo(on_wait=[w], on_update=[]))
                    e.engine = inst.engine
                    out.append(e)
                si.on_wait = waits[-lim:]
            out.append(inst)
        blk.instructions[:] = out
    return fix


def _audit_waits(nc):
    bad = []
    for blk in nc.m.functions[0].blocks:
        for inst in blk.instructions:
            lim = _wait_limit(inst)
            si = inst.sync_info
            nw = len(si.on_wait) if si and si.on_wait else 0
            if lim is not None and nw > lim:
                bad.append((type(inst).__name__, inst.name, nw))
    return bad


def _audit_streams(nc, audit):
    """The gpsimd stream must contain the CC triggers unobstructed: no
    instruction placed before the last trigger may wait on a semaphore
    that a CC op updates, and no readback/yout may sit on gpsimd at all.
    On sync, input loads must precede all readbacks."""
    def _unwrap(i):
        return getattr(i, "ins", i)

    bad = []
    names = {}
    for key, insts in audit.items():
        for i in insts:
            u = _unwrap(i)
            if u is not None and getattr(u, "name", None):
                names[u.name] = key
    cc_sems = set()
    for i in audit["cc"]:
        si = getattr(_unwrap(i), "sync_info", None)
        if si and si.on_update:
            for u in si.on_update:
                cc_sems.add(getattr(u, "semaphore", None) or str(u))
    cc_names = {_unwrap(i).name for i in audit["cc"]}

    streams = {}
    for blk in nc.m.functions[0].blocks:
        for inst in blk.instructions:
            streams.setdefault(str(inst.engine), []).append(inst)
    for eng, insts in streams.items():
        kinds = [names.get(i.name) for i in insts]
        is_gp = any(i.name in cc_names for i in insts)
        if is_gp:
            last_cc = max(j for j, i in enumerate(insts)
                          if i.name in cc_names)
            for j, i in enumerate(insts[:last_cc]):
                if kinds[j] in ("rb", "yout"):
                    bad.append(("cc-blocked-by", eng, i.name, kinds[j]))
                waits = list(i.sync_info.on_wait) if i.sync_info and \
                    i.sync_info.on_wait else []
                for wsem in waits:
                    s = getattr(wsem, "semaphore", None) or str(wsem)
                    if s in cc_sems:
                        bad.append(("cc-sem-wait-before-trigger", eng,
                                    i.name))
        if any(k == "rb" for k in kinds):
            first_rb = kinds.index("rb")
            for j in range(first_rb, len(insts)):
                if kinds[j] == "load":
                    bad.append(("load-after-rb", eng, insts[j].name))
    return bad


def build_nc_checked(max_tries=6):
    last = None
    for i in range(max_tries):
        nc, audit = build_nc(jitter=i)
        _split_excess_waits(nc)
        bad = _audit_waits(nc) + _audit_streams(nc, audit)
        if not bad:
            return nc
        last = bad
    raise RuntimeError(f"could not find wait-feasible schedule: {last[:5]}")


_CACHED = {}


def kernel(x, W_kqv):
    x = np.asarray(x, np.float32)
    W = np.asarray(W_kqv, np.float32)
    if "nc" not in _CACHED:
        _CACHED["nc"] = build_nc_checked()
    nc = _CACHED["nc"]
    in_maps = _host_inputs(x, W)
    res = run_bass_kernel_spmd(nc, in_maps, core_ids=list(range(8)))
    return _gather(res.results)


if __name__ == "__main__":
    x = np.random.randn(B, T, C).astype(np.float32)
    W = (np.random.randn(C, 3 * C) * 0.02).astype(np.float32)
    y = kernel(x, W)
    print("kernel ran:", y.shape, y.dtype)


# revision 20
# speedup vs baseline: 1.0368x; 1.0368x over previous
"""Causal self-attention (B=4,T=2048,C=1024) on 8 TRN2 NeuronCores.

Sharding: core c = 2*b + h handles batch b and global q-blocks g = 2k+h
(k=0..7, 128 rows each). K/V projection work is split across the pair:
core h projects kv only for s-block pairs (4r+2h, 4r+2h+1), r=0..3.
Pairs are exchanged with 3 pairwise AllGather rounds (DRAM bounce)
covering s-blocks 0..7 / 8..11 / 12..15; each round has ~15us fixed
cost on the CC core, so the first round is double-size.  The rounds
ride under the staging + q-projection compute and land (with
need-ordered readbacks) just before attention consumes them.

Engine roles are kept disjoint so the Tile scheduler cannot interleave
a CC-completion wait in front of a later CC trigger:
  gpsimd: wk/xT per-cb slab loads (2KB lines), snd DMAs, CC triggers.
  sync:   wv/wq/xq/mask loads, then per-round readbacks of rcv.
  scalar: PSUM drains, exp, y DMA posts.
  vector: attention probsT copies / reductions.
Attention runs q-blocks ascending with k=0 moved last (small final
tail); the only work depending on the last exchange round is the tail
of k=6/k=7, ~40us after the round completes.  Softmax skips
max-subtraction (scores are bounded ~2.5) and blocks are software-
pipelined (scores of block i+1 run between scores and AV of block i).
"""

import math
import sys

for p in ("/opt/trn_rl_repo",):
    if p not in sys.path:
        sys.path.insert(0, p)

import numpy as np
import ml_dtypes

import concourse.bass as bass
import concourse.tile as tile
from concourse import mybir
from concourse.masks import make_identity
from concourse.bass_utils import run_bass_kernel_spmd

B, T, C = 4, 2048, 1024
P = 128
NQB = 8            # q-blocks per core
NCB = C // P       # 8 c-chunks (contraction for projections)
NDB = C // P       # 8 d-chunks (contraction for QK)
NSB = T // P       # 16 s-blocks
F32 = mybir.dt.float32
BF16 = mybir.dt.bfloat16
SCALE = 1.0 / math.sqrt(C)
NEG = -1e30
GROUPS = [[0, 1], [2, 3], [4, 5], [6, 7]]
ROUNDS = [(0,), (1,), (2,), (3,)]   # local-pair grouping per exchange round


def build_nc(jitter=0):
    nc = bass.Bass(num_devices=8)
    xT = nc.declare_dram_parameter("xT", [C, T // 2], BF16, isOutput=False)
    xq = nc.declare_dram_parameter("xq", [C, NQB * P], BF16, isOutput=False)
    w = nc.declare_dram_parameter("w", [C, 3 * C], BF16, isOutput=False)
    mask = nc.declare_dram_parameter("mask", [P, 2 * P], BF16, isOutput=False)
    out = nc.declare_dram_parameter("out", [NQB * P, C], BF16, isOutput=True)

    audit = {"cc": [], "rb": [], "load": [], "yout": []}

    from contextlib import ExitStack
    with tile.TileContext(nc) as tc, ExitStack() as ctx:
        singles = ctx.enter_context(tc.tile_pool(name="singles", bufs=1))
        xqpool = ctx.enter_context(tc.tile_pool(name="xqpool", bufs=1))
        xtpool = ctx.enter_context(tc.tile_pool(name="xtpool", bufs=1))
        wbuf = ctx.enter_context(tc.tile_pool(name="wbuf", bufs=1))
        qkv = ctx.enter_context(tc.tile_pool(name="qkv", bufs=1))
        att = ctx.enter_context(tc.tile_pool(name="att", bufs=4))
        attT = ctx.enter_context(tc.tile_pool(name="attT", bufs=4))
        ybuf = ctx.enter_context(tc.tile_pool(name="ybuf", bufs=2))
        stat = ctx.enter_context(tc.tile_pool(name="stat", bufs=4))
        psA = ctx.enter_context(tc.tile_pool(name="psA", bufs=4, space="PSUM"))
        psT = ctx.enter_context(tc.tile_pool(name="psT", bufs=2, space="PSUM"))
        psY = ctx.enter_context(tc.tile_pool(name="psY", bufs=2, space="PSUM"))
        dram = ctx.enter_context(tc.tile_pool(name="dram", bufs=10,
                                              space="DRAM"))

        ident = singles.tile([P, P], BF16)
        make_identity(nc, ident)

        touch_scr = stat.tile([P, 2], F32, tag="touch")
        for _ in range(jitter):  # schedule perturbation for wait-audit retries
            nc.vector.tensor_copy(out=touch_scr, in_=touch_scr)

        # resident weights / activations; DMA issue order = need order.
        wk_all = wbuf.tile([P, NCB, C], BF16, tag="wk_all")
        wv_all = wbuf.tile([P, NCB, C], BF16, tag="wv_all")
        wq_all = wbuf.tile([P, NCB, C], BF16, tag="wq_all")
        xT_sb = xtpool.tile([P, NCB, T // 2], BF16, tag="xT")
        xq_sb = xqpool.tile([P, NCB, NQB * P], BF16, tag="xq")
        mask_sb = singles.tile([P, 2 * P], BF16)

        # Split the load bandwidth across the two DMA queues in first-need
        # order: gpsimd carries wk slabs then wv (K then V projection
        # inputs), sync carries xT slabs then wq/xq.  All 2KB lines.
        for cb in range(NCB):
            audit["load"].append(nc.gpsimd.dma_start(
                out=wk_all[:, cb, :], in_=w[cb * P:(cb + 1) * P, C:2 * C]))
            audit["load"].append(nc.sync.dma_start(
                out=xT_sb[:, cb, :], in_=xT[cb * P:(cb + 1) * P, :]))
        audit["load"].append(nc.sync.dma_start(
            out=wv_all,
            in_=w[:, 2 * C:3 * C].rearrange("(cb p) d -> p cb d", p=P)))
        audit["load"].append(nc.sync.dma_start(
            out=wq_all, in_=w[:, 0:C].rearrange("(cb p) d -> p cb d", p=P)))
        audit["load"].append(nc.sync.dma_start(
            out=xq_sb, in_=xq[:, :].rearrange("(cb p) t -> p cb t", p=P)))
        audit["load"].append(nc.sync.dma_start(out=mask_sb, in_=mask[:, :]))

        # persistent SBUF tensors
        qT_sb = qkv.tile([P, NDB, NQB * P], BF16)   # [d%128, d//128, t]  2MB
        kT_sb = qkv.tile([P, NDB, T], BF16)         # [d%128, d//128, s]  4MB
        v_sb = qkv.tile([P, NSB, C], BF16)          # [s%128, s//128, d]  4MB

        # ---------------- Phase KV + exchange -----------------------------
        # Local pair r (xT_sb cols 256r..256r+256) is staged at the slot-0
        # position of its round: kT_sb cols 512r..512r+256 / v_sb blocks
        # (4r, 4r+1).  The round's AllGather then lands slot h -> cols
        # 512r+256h..512r+256(h+1) / v blocks (4r+2h, 4r+2h+1), making
        # every kT/v region rb-written and final in global s order.
        deferred_rb = []
        for prs in ROUNDS:
            npair = len(prs)
            for db in range(NDB):
                ps = psA.tile([P, 512], F32, tag="ps")
                wd = npair * 2 * P
                xo = prs[0] * 2 * P
                for cb in range(NCB):
                    nc.tensor.matmul(
                        ps[:, 0:wd], wk_all[:, cb, db * P:(db + 1) * P],
                        xT_sb[:, cb, xo:xo + wd],
                        start=(cb == 0), stop=(cb == NCB - 1))
                for i, pr in enumerate(prs):
                    nc.scalar.copy(
                        out=kT_sb[:, db, 4 * pr * P:(4 * pr + 2) * P],
                        in_=ps[:, 2 * i * P:2 * (i + 1) * P])
            for pr in prs:
                for i, sblk in enumerate((4 * pr, 4 * pr + 1)):
                    ps0 = psA.tile([P, 512], F32, tag="ps")
                    ps1 = psA.tile([P, 512], F32, tag="ps")
                    xc = pr * 2 * P + i * P
                    for cb in range(NCB):
                        nc.tensor.matmul(
                            ps0, xT_sb[:, cb, xc:xc + P],
                            wv_all[:, cb, 0:512],
                            start=(cb == 0), stop=(cb == NCB - 1))
                        nc.tensor.matmul(
                            ps1, xT_sb[:, cb, xc:xc + P],
                            wv_all[:, cb, 512:1024],
                            start=(cb == 0), stop=(cb == NCB - 1))
                    nc.scalar.copy(out=v_sb[:, sblk, 0:512], in_=ps0)
                    nc.scalar.copy(out=v_sb[:, sblk, 512:1024], in_=ps1)

            # exchange round: snd = [kT(pair) for pair] + [v(pair) for pair]
            slot = npair * 4096          # bf16 elements per partition
            snd = dram.tile([P, slot], BF16, tag=f"snd{prs[0]}")
            rcv = dram.tile([2 * P * slot], BF16, tag=f"rcv{prs[0]}")
            for i, pr in enumerate(prs):
                nc.gpsimd.dma_start(
                    out=snd[:, i * 2048:(i + 1) * 2048],
                    in_=kT_sb[:, :, 4 * pr * P:(4 * pr + 2) * P])
            for i, pr in enumerate(prs):
                nc.gpsimd.dma_start(
                    out=snd[:, (npair + i) * 2048:(npair + i + 1) * 2048],
                    in_=v_sb[:, 4 * pr:4 * pr + 2, :])
            cc = nc.gpsimd.collective_compute(
                "AllGather", mybir.AluOpType.bypass, replica_groups=GROUPS,
                ins=[snd[:, :].rearrange("p n -> (p n)")],
                outs=[rcv[:]])
            audit["cc"].append(cc)

            def rb(prs=prs, npair=npair, slot=slot, rcv=rcv):
                # need-ordered: kT of both slots first, then v of both
                # slots, earliest s-blocks first.
                saps = [rcv[s * P * slot:(s + 1) * P * slot].rearrange(
                    "(p c h) -> p c h", p=P, c=2 * npair) for s in range(2)]
                for i, pr in enumerate(prs):
                    for s in range(2):
                        co = (4 * pr + 2 * s) * P
                        audit["rb"].append(nc.sync.dma_start(
                            out=kT_sb[:, :, co:co + 2 * P],
                            in_=saps[s][:, i, :]))
                for i, pr in enumerate(prs):
                    for s in range(2):
                        audit["rb"].append(nc.sync.dma_start(
                            out=v_sb[:, 4 * pr + 2 * s:4 * pr + 2 * s + 2,
                                     :],
                            in_=saps[s][:, npair + i, :]))
            deferred_rb.append(rb)

        # all CC triggers posted; now post the readbacks (sync engine, so a
        # readback waiting on a CC-completion semaphore can never delay a
        # later trigger on the gpsimd stream)
        for rb_fn in deferred_rb:
            rb_fn()

        # ---------------- Phase Q: qT = (W_q^T @ xq) * scale --------------
        for th in range(2):
            for db in range(NDB):
                ps = psA.tile([P, 512], F32, tag="ps")
                for cb in range(NCB):
                    nc.tensor.matmul(
                        ps, wq_all[:, cb, db * P:(db + 1) * P],
                        xq_sb[:, cb, th * 512:(th + 1) * 512],
                        start=(cb == 0), stop=(cb == NCB - 1))
                nc.scalar.mul(
                    out=qT_sb[:, db, th * 512:(th + 1) * 512], in_=ps,
                    mul=SCALE)

        # ---------------- Phase ATT (software pipelined) ------------------
        # Unit schedule (S_k = scores of q-block k, split into 512-col
        # chunks where useful; T_k = transpose+AV tail; X_k = early
        # transposes):
        #   S1 | S0 T1 | S2 T0 | S3 T2 | S7a T3 | S4 X7 | S5 T4
        #   | S6a T5 | X6 S7b S6b | T7 | T6
        # S0/S7a need only s-blocks 0..7 and S6a only 0..11, so the only
        # work gated on the last exchange rounds sits at the very end.
        state = {}

        def emit_scores(k, chlo, chhi):
            L = 2 * k + 2
            cols = L * P
            nch = (cols + 511) // 512
            if k not in state:
                state[k] = {
                    "probs": att.tile([P, NQB * 2 * P], BF16, tag="probs",
                                      name=f"probs{k}"),
                    "sums": stat.tile([P, 8], F32, tag="sums",
                                      name=f"sums{k}"),
                    "probsT": None, "nch": nch,
                }
            st = state[k]
            lo = cols - 256
            ch0, off = divmod(lo, 512)
            for ch in range(chlo, min(chhi, nch)):
                wd = min(512, cols - ch * 512)
                ps = psA.tile([P, 512], F32, tag="ps")
                has_mask = ch == ch0
                for db in range(NDB):
                    nc.tensor.matmul(
                        ps[:, 0:wd], qT_sb[:, db, k * P:(k + 1) * P],
                        kT_sb[:, db, ch * 512:ch * 512 + wd],
                        start=(db == 0),
                        stop=(not has_mask and db == NDB - 1))
                if has_mask:
                    # mask folded into the accumulation group
                    nc.tensor.matmul(
                        ps[:, off:off + 256], ident, mask_sb,
                        start=False, stop=True)
                # no max-subtraction: |score| <= ~2.5, exp is safe
                nc.scalar.activation(
                    out=st["probs"][:, ch * 512:ch * 512 + wd],
                    in_=ps[:, 0:wd],
                    func=mybir.ActivationFunctionType.Exp,
                    bias=0.0, scale=1.0,
                    accum_out=st["sums"][:, ch:ch + 1])

        def emit_transp(k, jlo, jhi):
            st = state[k]
            if st["probsT"] is None:
                st["probsT"] = attT.tile([P, NQB * 2, P], BF16, tag="probsT",
                                         name=f"probsT{k}")
            probs, probsT = st["probs"], st["probsT"]
            for j in range(jlo, jhi, 4):
                g = min(4, jhi - j)
                pt = psT.tile([P, 4 * P], BF16)
                for jj in range(g):
                    nc.tensor.transpose(
                        pt[:, jj * P:(jj + 1) * P],
                        probs[:, (j + jj) * P:(j + jj + 1) * P], ident)
                nc.vector.tensor_copy(out=probsT[:, j:j + g, :],
                                      in_=pt[:, 0:g * P])

        def emit_av(k):
            L = 2 * k + 2
            st = state[k]
            probsT = st["probsT"]
            rsum = stat.tile([P, 1], F32, tag="rsum")
            nc.vector.reduce_sum(
                out=rsum, in_=st["sums"][:, 0:st["nch"]],
                axis=mybir.AxisListType.X)
            recip = stat.tile([P, 1], F32, tag="recip")
            nc.vector.reciprocal(out=recip, in_=rsum)
            y_sb = ybuf.tile([P, C], BF16, tag="y")
            for dh in range(2):
                py = psY.tile([P, 512], F32, tag="py")
                for j in range(L):
                    nc.tensor.matmul(
                        py, probsT[:, j, :],
                        v_sb[:, j, dh * 512:(dh + 1) * 512],
                        start=(j == 0), stop=(j == L - 1))
                nc.scalar.activation(
                    out=y_sb[:, dh * 512:(dh + 1) * 512], in_=py,
                    func=mybir.ActivationFunctionType.Copy, bias=0.0,
                    scale=recip)
            audit["yout"].append(nc.scalar.dma_start(
                out=out[k * P:(k + 1) * P, :], in_=y_sb))

        def tail(k):
            emit_transp(k, 0, 2 * k + 2)
            emit_av(k)

        emit_scores(1, 0, 4)
        emit_scores(0, 0, 4); tail(1)
        emit_scores(2, 0, 4); tail(0)
        emit_scores(3, 0, 4); tail(2)
        emit_scores(7, 0, 2); tail(3)           # k7 cols 0:1024 (s0..7)
        emit_scores(4, 0, 4); emit_transp(7, 0, 8)
        emit_scores(5, 0, 4); tail(4)
        emit_scores(6, 0, 3); tail(5)           # k6 cols 0:1536 (s0..11)
        emit_transp(6, 0, 8)
        emit_scores(7, 2, 4)                    # k7 cols 1024:2048
        emit_transp(6, 8, 12)
        emit_scores(6, 3, 4)                    # k6 cols 1536:1792
        emit_transp(7, 8, 16); emit_av(7)
        emit_transp(6, 12, 14); emit_av(6)

    return nc, audit


def _host_inputs(x, W):
    """Build per-core input maps."""
    tril = np.where(
        np.arange(P)[None, :] <= np.arange(P)[:, None], 0.0, NEG
    ).astype(np.float32)
    mask_even = np.concatenate([tril, np.full((P, P), NEG, np.float32)], 1)
    mask_odd = np.concatenate([np.zeros((P, P), np.float32), tril], 1)
    in_maps = []
    for c in range(8):
        b, h = divmod(c, 2)
        xb = x[b].astype(ml_dtypes.bfloat16)        # [T, C]
        # local kv token columns: pairs (4r+2h, 4r+2h+1), r=0..3
        kvrows = np.concatenate(
            [np.arange((4 * r + 2 * h) * P, (4 * r + 2 * h + 2) * P)
             for r in range(4)])
        xTh = np.ascontiguousarray(xb[kvrows].T)    # [C, 1024]
        qrows = np.concatenate(
            [np.arange((2 * k + h) * P, (2 * k + h + 1) * P)
             for k in range(NQB)])
        xq = np.ascontiguousarray(xb[qrows].T)      # [C, 1024]
        in_maps.append({
            "xT": xTh, "xq": xq, "w": W.astype(ml_dtypes.bfloat16),
            "mask": (mask_even if h == 0 else mask_odd).astype(
                ml_dtypes.bfloat16),
        })
    return in_maps


def _gather(results):
    y = np.zeros((B, T, C), np.float32)
    for c in range(8):
        b, h = divmod(c, 2)
        yc = results[c]["out"]
        for k in range(NQB):
            g = 2 * k + h
            y[b, g * P:(g + 1) * P, :] = yc[k * P:(k + 1) * P, :]
    return y


_SKIP_TYPES = ("InstCall", "InstUnconditionalBranch")


def _wait_limit(inst):
    t = type(inst).__name__
    if t in _SKIP_TYPES:
        return None
    return 1


def _split_excess_waits(nc):
    """HW instruction structs carry few sync-wait slots (1 for compute,
    2 for pseudo-DMA). Move excess waits onto same-engine EventSemaphore
    instructions inserted just before the offender (engines execute their
    stream in order, so this preserves semantics)."""
    fix = 0
    for blk in nc.m.functions[0].blocks:
        out = []
        for inst in blk.instructions:
            lim = _wait_limit(inst)
            si = inst.sync_info
            waits = list(si.on_wait) if si and si.on_wait else []
            if lim is not None and len(waits) > lim:
                for w in waits[:-lim]:
                    fix += 1
                    e = mybir.InstEventSemaphore(
                        name=f"I-waitfix-{fix}", ins=[], outs=[],
                        sync_info=mybir.SyncInfo(on_wait=[w], on_update=[]))
                    e.engine = inst.engine
                    out.append(e)
                si.on_wait = waits[-lim:]
            out.append(inst)
        blk.instructions[:] = out
    return fix


def _audit_waits(nc):
    bad = []
    for blk in nc.m.functions[0].blocks:
        for inst in blk.instructions:
            lim = _wait_limit(inst)
            si = inst.sync_info
            nw = len(si.on_wait) if si and si.on_wait else 0
            if lim is not None and nw > lim:
                bad.append((type(inst).__name__, inst.name, nw))
    return bad


def _audit_streams(nc, audit):
    """The gpsimd stream must contain the CC triggers unobstructed: no
    instruction placed before the last trigger may wait on a semaphore
    that a CC op updates, and no readback/yout may sit on gpsimd at all.
    On sync, input loads must precede all readbacks."""
    def _unwrap(i):
        return getattr(i, "ins", i)

    bad = []
    names = {}
    for key, insts in audit.items():
        for i in insts:
            u = _unwrap(i)
            if u is not None and getattr(u, "name", None):
                names[u.name] = key
    cc_sems = set()
    for i in audit["cc"]:
        si = getattr(_unwrap(i), "sync_info", None)
        if si and si.on_update:
            for u in si.on_update:
                cc_sems.add(getattr(u, "semaphore", None) or str(u))
    cc_names = {_unwrap(i).name for i in audit["cc"]}

    streams = {}
    for blk in nc.m.functions[0].blocks:
        for inst in blk.instructions:
            streams.setdefault(str(inst.engine), []).append(inst)
    for eng, insts in streams.items():
        kinds = [names.get(i.name) for i in insts]
        is_gp = any(i.name in cc_names for i in insts)
        if is_gp:
            last_cc = max(j for j, i in enumerate(insts)
                          if i.name in cc_names)
            for j, i in enumerate(insts[:last_cc]):
                if kinds[j] in ("rb", "yout"):
                    bad.append(("cc-blocked-by", eng, i.name, kinds[j]))
                waits = list(i.sync_info.on_wait) if i.sync_info and \
                    i.sync_info.on_wait else []
                for wsem in waits:
                    s = getattr(wsem, "semaphore", None) or str(wsem)
                    if s in cc_sems:
                        bad.append(("cc-sem-wait-before-trigger", eng,
                                    i.name))
        if any(k == "rb" for k in kinds):
            first_rb = kinds.index("rb")
            for j in range(first_rb, len(insts)):
                if kinds[j] == "load":
                    bad.append(("load-after-rb", eng, insts[j].name))
    return bad


def build_nc_checked(max_tries=6):
    last = None
    for i in range(max_tries):
        nc, audit = build_nc(jitter=i)
        _split_excess_waits(nc)
        bad = _audit_waits(nc) + _audit_streams(nc, audit)
        if not bad:
            return nc
        last = bad
    raise RuntimeError(f"could not find wait-feasible schedule: {last[:5]}")


_CACHED = {}


def kernel(x, W_kqv):
    x = np.asarray(x, np.float32)
    W = np.asarray(W_kqv, np.float32)
    if "nc" not in _CACHED:
        _CACHED["nc"] = build_nc_checked()
    nc = _CACHED["nc"]
    in_maps = _host_inputs(x, W)
    res = run_bass_kernel_spmd(nc, in_maps, core_ids=list(range(8)))
    return _gather(res.results)


if __name__ == "__main__":
    x = np.random.randn(B, T, C).astype(np.float32)
    W = (np.random.randn(C, 3 * C) * 0.02).astype(np.float32)
    y = kernel(x, W)
    print("kernel ran:", y.shape, y.dtype)
